# revision 50
# baseline (speedup 1.0000x reference)
"""Trainium2 Bass kernel for nn_Attention_21895743275585.

Reference computation (per batch b of 4):
  qkv = w_qkv @ x_flat            # 1x1 conv, x_flat [C=256, N=2304]
  q,k l2-normalized per (head, n) along dim_head=64; SCALE=10
  sim = 10 * qhat^T khat per head; attn = softmax(sim, axis=-1)
  out = attn @ v; final = w_out @ out_inner + b_out

Sharding: 8 cores = (batch b, head-half). Each core handles 4 of the 8 heads
of one batch; on-core the two head-pair output projections accumulate in
PSUM, so each core emits one [256, N] partial and the host sums the two
half-cores per batch (bias fed only to half 0).

On-core layout ([partition, free]):
  q,k "channels-major" [d, n]: qk4 [128, 4(t), N] with t in {q01,k01,q23,k23}
  v transposed [n, d] per j-tile with a ones column appended (65-wide
  stationary), so each E@v matmul also emits that head's softmax
  denominator row into PSUM partition 64 -- no separate ones matmuls.
  sim^T chunk [j, i] = k^T q; sims are emitted in groups of two j-tiles:
  consecutive sim matmuls stream moving data from alternating SBUF
  partition halves (q rows 0:64 / 64:128), which the PE overlaps; the
  four E@v matmuls of the group follow (full-partition moving, serial).
  softmax exp PSUM->SBUF splits between ACT (table Exp) and DVE (one-op
  Schraudolph: int16(x*184.665+16251) bitcast to bf16; numerator and
  denominator share the approximation so its ~3% jitter cancels in the
  softmax). 1/sqrt for the l2 norms runs as exp(-0.5 ln x) on ACT (one
  pinned table set); norm rows broadcast across partitions on the GPSIMD
  engine (no DRAM bounce), then one 2x-mode bf16 DVE multiply per (pair,
  chunk) forms qhat,khat packed [128, 4, N]. GPSIMD also takes the big
  memsets and the pair-1 squares/multiplies that sit inside the
  head-pair-0 attention window.
  Phase 1 (projection chunks 1-4 + v^T tiles) is interleaved INTO the
  first attention chunk's j-loop: the PE executes its queue in order, so
  emitting sims early keeps every engine fed during the latency-bound
  projection->norm chains. All projection work shares one PSUM pool
  (PSSIM 4 banks + PSO 2 + PSF 2 = 8).
"""

import math

import numpy as np

B, C, H, W = 4, 256, 48, 48
HEADS, DIM_HEAD, SCALE = 8, 64, 10.0
INNER = HEADS * DIM_HEAD
N = H * W                      # 2304
NJ = N // 128                  # 18 j-tiles
CHUNKS = [(0, 512), (512, 512), (1024, 512), (1536, 512), (2048, 256)]
EPS = 1e-12

WD_NAME = "bf16"               # working dtype: "bf16" | "f32r" | "f32"

# softmax-exp engine split: these j-tiles run on DVE (one-op Schraudolph),
# the rest on ACT (table exp). Tuned so ACT ~ DVE ~ just under PE per chunk.
DVE_JTS = {0: (1, 5, 8, 10, 12, 14, 16), 1: (2, 5, 8, 10, 12, 14, 16)}
LOG2E = 1.4426950408889634
# fp32-bits Schraudolph constants (legacy two-op path, kept registered)
A_EXP = float(2 ** 23) * LOG2E
B_EXP = float(127 * 2 ** 23) + 0.5
MASK_C = float(np.int32(0x007FFFFF).view(np.float32))
GAMMA = 0.235
# bf16-bits one-op Schraudolph: int16(x * 128*log2e + (127*128 + 0.5 - 5.51))
# the -5.51 centers the (1+f)/2^f interpolation error to +-3.0%.
A_E16 = 128.0 * LOG2E
B_E16 = 127.0 * 128.0 + 0.5 - 5.5085

_CACHE = {}


def _register_exp_ops():
    """Register the Schraudolph exp ops into concourse's custom-DVE tables
    (runtime registration; shas computed on the fly)."""
    import concourse.dve_ops as dops
    if "EXP_B16_ANT" in dops.CUSTOM_DVE_SPECS:
        return {"bits": dops._EXP_BITS_ANT, "fix": dops._EXP_FIX_ANT,
                "b16": dops._EXP_B16_ANT}
    from concourse.dve_spec import Spec, Src0, C0, C1, C2, AluOp, Bin, lower
    from concourse.dve_uop import DveOpSpec
    from concourse.dve_ops import DveOp

    def mk(name, spec):
        shas = {}
        for ver in ("v3", "v4"):
            try:
                sp = DveOpSpec(name=name, opcode=1,
                               uops=lower(spec, ver=ver), rd1_en=False)
                shas[ver] = sp.sha(ver)
            except Exception:
                pass
        op = DveOp(name, spec, subdim=False, uops_sha=shas)
        row = max(dops._SUB_OPCODE_FOR_NAME.values()) + 1
        assert row < 0x20
        dops.OPS.append(op)
        dops._SUB_OPCODE_FOR_NAME[op.name] = row
        dops.CUSTOM_DVE_SPECS[op.name] = op.spec
        return op

    def ref_bits(in0, in1, c0, c1, c2):
        t = in0.astype(np.float32) * np.float32(c0) + np.float32(c1)
        return t.astype(np.int32)

    spec_bits = Spec(body=Src0 * C0 + C1, reference=ref_bits)

    _and = Bin(AluOp.BITWISE_AND, Src0, C0)
    _u = Bin(AluOp.BITWISE_OR, _and, C1)
    _f = _u - C1
    _c = _f * (C1 - _f) * C2 + C1

    def ref_fix(in0, in1, c0, c1, c2):
        bits = np.asarray(in0, np.float32).view(np.int32)
        m = bits & 0x007FFFFF
        u = (m | 0x3F800000).astype(np.int32).view(np.float32)
        f = u - np.float32(c1)
        c = f * (np.float32(c1) - f) * np.float32(c2) + np.float32(c1)
        return np.asarray(in0, np.float32).view(np.float32) * c

    spec_fix = Spec(body=Src0 * _c, reference=ref_fix)

    def ref_b16(in0, in1, c0, c1, c2):
        # hardware: fp32 ALU result, output-stage convert to int16 (trunc);
        # CoreSim casts the returned float to the out AP dtype itself
        t = in0.astype(np.float32) * np.float32(c0) + np.float32(c1)
        return t

    spec_b16 = Spec(body=Src0 * C0 + C1, reference=ref_b16)

    dops._EXP_BITS_ANT = mk("EXP_BITS_ANT", spec_bits)
    dops._EXP_FIX_ANT = mk("EXP_FIX_ANT", spec_fix)
    dops._EXP_B16_ANT = mk("EXP_B16_ANT", spec_b16)
    return {"bits": dops._EXP_BITS_ANT, "fix": dops._EXP_FIX_ANT,
            "b16": dops._EXP_B16_ANT}


def _pin_act_tables():
    """Force every activation onto the natural_log_exp_and_others set so the
    whole kernel needs exactly one ACT table load (Ln+Exp share that set)."""
    import concourse.bacc as bacc_mod
    if getattr(bacc_mod, "_act_tables_pinned", False):
        return
    orig = bacc_mod.get_activation_tables

    def patched(arch):
        t = orig(arch)
        keep = "natural_log_exp_and_others"
        if keep in t:
            return {name: (funcs if name == keep else set())
                    for name, funcs in t.items()}
        return t

    bacc_mod.get_activation_tables = patched
    bacc_mod._act_tables_pinned = True


def _build(wd_name):
    import concourse.bass as bass
    import concourse.tile as tile
    from concourse import bacc, mybir

    _pin_act_tables()
    expops = _register_exp_ops()

    F32 = mybir.dt.float32
    I16 = mybir.dt.int16
    F32R = mybir.dt.float32r
    WD = mybir.dt.bfloat16 if wd_name == "bf16" else F32

    def mc(ap):
        # matmul operand cast for the fast-fp32 PE path
        return ap.bitcast(F32R) if wd_name == "f32r" else ap

    Ln = mybir.ActivationFunctionType.Ln
    Exp = mybir.ActivationFunctionType.Exp
    ActCopy = mybir.ActivationFunctionType.Copy

    nc = bacc.Bacc("TRN2", target_bir_lowering=False, debug=False,
                   enable_asserts=False, num_devices=8)
    x2 = nc.dram_tensor("x2", [2, 128, N], WD, kind="ExternalInput").ap()
    wqk = nc.dram_tensor("wqk", [2, 128, 512], WD, kind="ExternalInput").ap()
    wvT = nc.dram_tensor("wvT", [2, 128, 256], WD, kind="ExternalInput").ap()
    woT = nc.dram_tensor("woT", [2, 128, 256], WD, kind="ExternalInput").ap()
    bias = nc.dram_tensor("bias", [2, 128, 1], F32, kind="ExternalInput").ap()
    ones8 = nc.dram_tensor("ones8", [128, 72], WD, kind="ExternalInput").ap()
    # output: both head-pair projections pre-summed in PSUM; host adds the
    # two half-cores per batch. [m2, 128, N] row-blocks of out channels.
    y = nc.dram_tensor("y", [2, 128, N], WD, kind="ExternalOutput").ap()
    # internal DRAM bounce rows for the partition broadcasts (GPSIMD
    # partition_broadcast miscomputes on hardware here, so both the norm
    # and scale paths bounce through DRAM)
    rsd = nc.dram_tensor("rsd", [8, N], WD, kind="Internal").ap()
    rsdd = nc.dram_tensor("rsdd", [4, N], F32, kind="Internal").ap()

    def bcast_row(dram_row_ap, dst_ap, parts):
        src = bass.AP(tensor=dram_row_ap.tensor, offset=dram_row_ap.offset,
                      ap=[[0, parts]] + list(dram_row_ap.ap))
        nc.sync.dma_start(dst_ap, src)

    # pair-major tile order t: 0=q01, 1=k01, 2=q23, 3=k23.
    # wqk stationary column block for t:  m = [0, 2, 1, 3][t]
    T2M = [0, 2, 1, 3]

    with tile.TileContext(nc) as tc:
        with tc.tile_pool(name="persist", bufs=1) as P, \
             tc.tile_pool(name="bcast", bufs=2) as RSB, \
             tc.tile_pool(name="sq", bufs=3) as SQ, \
             tc.tile_pool(name="esb", bufs=12) as ESB, \
             tc.tile_pool(name="yst", bufs=3) as YST, \
             tc.tile_pool(name="pssim", bufs=2, space="PSUM") as PSSIM, \
             tc.tile_pool(name="pso", bufs=1, space="PSUM") as PSO, \
             tc.tile_pool(name="psf", bufs=2, space="PSUM") as PSF:

            # ---- persistent tiles ----
            x_sb = [P.tile([128, N], WD, tag=f"x{c}", name=f"x{c}")
                    for c in range(2)]
            wqk_sb = [P.tile([128, 512], WD, tag=f"wqk{c}", name=f"wqk{c}")
                      for c in range(2)]
            wvT_sb = [P.tile([128, 256], WD, tag=f"wvT{c}", name=f"wvT{c}")
                      for c in range(2)]
            woT_sb = [P.tile([128, 256], WD, tag=f"woT{c}", name=f"woT{c}")
                      for c in range(2)]
            bias_sb = [P.tile([128, 1], F32, tag=f"bias{c}", name=f"bias{c}")
                       for c in range(2)]
            ones8_sb = P.tile([128, 72], WD, tag="ones8", name="ones8")

            # PE warmup: two matmuls on a memset tile ramp the PE clock out
            # of its low-power state while the input DMAs are in flight.
            wu = P.tile([128, 64], WD, tag="wu", name="wu")
            nc.vector.memset(wu[:, :], 0.25)
            for _ in range(2):
                pwu = PSF.tile([128, 512], F32, tag="pf", name="pwu")
                nc.tensor.matmul(pwu[0:64, 0:64], mc(wu[:, :]),
                                 mc(wu[:, 0:64]), start=True, stop=True)

            # chunked input DMA: qk weights + chunk 0 of x first so the
            # first projection matmul can start early.
            nc.sync.dma_start(wqk_sb[0][:, :], wqk[0])
            for c in range(2):
                nc.sync.dma_start(x_sb[c][:, 0:256], x2[c][:, 0:256])
            for c in range(2):
                nc.sync.dma_start(x_sb[c][:, 256:512], x2[c][:, 256:512])
            nc.sync.dma_start(wqk_sb[1][:, :], wqk[1])
            for c in range(2):
                nc.sync.dma_start(wvT_sb[c][:, :], wvT[c])
            for (off, cw) in CHUNKS[1:]:
                for c in range(2):
                    nc.sync.dma_start(x_sb[c][:, off:off + cw],
                                      x2[c][:, off:off + cw])
            for c in range(2):
                nc.sync.dma_start(woT_sb[c][:, :], woT[c])
                nc.sync.dma_start(bias_sb[c][:, :], bias[c])
            nc.sync.dma_start(ones8_sb[:, :], ones8)

            # per-partition Exp bias: ln(SCALE) on q rows (bases 0, 64),
            # 0 on k rows (bases 32, 96)
            biasln = P.tile([128, 1], F32, tag="biasln", name="biasln")
            nc.vector.memset(biasln[0:32, :], math.log(SCALE))
            nc.vector.memset(biasln[32:64, :], 0.0)
            nc.vector.memset(biasln[64:96, :], math.log(SCALE))
            nc.vector.memset(biasln[96:128, :], 0.0)

            qk4 = P.tile([128, 4, N], WD, tag="qk4", name="qk4")
            ss8 = P.tile([128, N], F32, tag="ss8", name="ss8")
            rs8 = P.tile([128, N], WD, tag="rs8", name="rs8")
            nc.gpsimd.memset(ss8[:, :], 1.0)

            qh4 = P.tile([128, 4, N], WD, tag="qh4", name="qh4")
            vT_sb = P.tile([128, NJ, 4, 65], WD, tag="vT", name="vT")
            nc.gpsimd.memset(vT_sb[:, :, :, 64:65], 1.0)

            numer = [P.tile([128, N], WD, tag=f"nu{p}", name=f"nu{p}")
                     for p in range(2)]
            nsc = [P.tile([128, N], WD, tag=f"nsc{p}", name=f"nsc{p}")
                   for p in range(2)]
            # softmax denominators: head rows at partitions 0 and 32
            s8 = P.tile([64, N], F32, tag="s8", name="s8")
            s8b = P.tile([64, N], F32, tag="s8b", name="s8b")
            rsden8 = P.tile([64, N], F32, tag="rsden", name="rsden")
            rsdenb = P.tile([64, N], WD, tag="rsdenb", name="rsdenb")

            def qhat(p):
                return qh4[:, 2 * p, :]

            def khat(p):
                return qh4[:, 2 * p + 1, :]

            # ---- projection / norm / v^T building blocks (PSF pool) ----
            # q2 squares live per chunk until the deferred norm-sum pass;
            # they borrow the e-tile slots (same 2KB size, disjoint lifetime)
            q2p0 = [ESB.tile([128, 2, 512], WD, tag="e",
                             name=f"q2p{ci}") for ci in range(len(CHUNKS))]

            def qkv_proj(p, off, cw, copy_eng, q2, PQ, ptag):
                """project q and k tiles of pair p for one chunk + square.
                The norm-sum matmuls are deferred (ss_sum) so the PE queue
                never stalls on the drain->square chain."""
                for ti in range(2):
                    t = 2 * p + ti
                    m = T2M[t]
                    pq = PQ.tile([128, 512], F32, tag=ptag, name="pq")
                    for c in range(2):
                        nc.tensor.matmul(
                            pq[:, 0:cw],
                            mc(wqk_sb[c][:, m * 128:(m + 1) * 128]),
                            mc(x_sb[c][:, off:off + cw]),
                            start=(c == 0), stop=(c == 1))
                    if copy_eng == "act":
                        nc.scalar.activation(qk4[:, t, off:off + cw],
                                             pq[:, 0:cw], ActCopy)
                    else:
                        nc.vector.tensor_copy(qk4[:, t, off:off + cw],
                                              pq[:, 0:cw])
                sq_eng = nc.gpsimd
                sq_eng.tensor_mul(q2[:, :, 0:cw],
                                  qk4[:, 2 * p:2 * p + 2, off:off + cw],
                                  qk4[:, 2 * p:2 * p + 2, off:off + cw])

            def ss_sum(p, off, cw, copy_eng, q2, PQ, ptag):
                for ti in range(2):
                    base = 32 * (2 * p + ti)
                    pss = PQ.tile([128, 512], F32, tag=ptag, name="pss")
                    nc.tensor.matmul(pss[0:2, 0:cw],
                                     mc(ones8_sb[:, 0:2]),
                                     mc(q2[:, ti, 0:cw]),
                                     start=True, stop=True)
                    if copy_eng == "act":
                        nc.scalar.activation(
                            ss8[base:base + 2, off:off + cw],
                            pss[0:2, 0:cw], ActCopy)
                    else:
                        nc.vector.tensor_copy(
                            ss8[base:base + 2, off:off + cw],
                            pss[0:2, 0:cw])

            def qkv_pair(p, off, cw, copy_eng):
                q2 = SQ.tile([128, 2, 512], WD, tag="q2", name="q2")
                qkv_proj(p, off, cw, copy_eng, q2, PSF, "pf")
                ss_sum(p, off, cw, copy_eng, q2, PSF, "pf")

            def rs_chunk(p, off, cw):
                # rs = exp(-0.5 ln(ss) + biasln) on the packed norm rows
                b0 = 64 * p
                sl = slice(b0, b0 + 34)
                lnq = SQ.tile([64, 512], F32, tag="lnq", name="lnq")
                nc.scalar.activation(lnq[0:34, 0:cw], ss8[sl, off:off + cw],
                                     Ln)
                nc.scalar.activation(rs8[sl, off:off + cw],
                                     lnq[0:34, 0:cw], Exp,
                                     scale=-0.5, bias=biasln[sl, :])
                for a in (2 * p, 2 * p + 1):
                    nc.sync.dma_start(rsd[2 * a:2 * a + 2, off:off + cw],
                                      rs8[32 * a:32 * a + 2, off:off + cw])

            def norm_chunk(p, off, cw, rsbp, mul_eng):
                # rsbp [128, 2, N] bf16: [:,0,:] q-norm rows, [:,1,:] k
                for ti in range(2):
                    a = 2 * p + ti
                    bcast_row(rsd[2 * a][off:off + cw],
                              rsbp[0:64, ti, off:off + cw], 64)
                    bcast_row(rsd[2 * a + 1][off:off + cw],
                              rsbp[64:128, ti, off:off + cw], 64)
                mul_eng.tensor_mul(qh4[:, 2 * p:2 * p + 2, off:off + cw],
                                   qk4[:, 2 * p:2 * p + 2, off:off + cw],
                                   rsbp[:, :, off:off + cw])

            def vproj(jt):
                pv = PSF.tile([128, 512], F32, tag="pf", name="pv")
                for c in range(2):
                    nc.tensor.matmul(
                        pv[:, 0:256],
                        mc(x_sb[c][:, jt * 128:(jt + 1) * 128]),
                        mc(wvT_sb[c][:, :]),
                        start=(c == 0), stop=(c == 1))
                if jt % 3 != 1:
                    nc.vector.tensor_copy(
                        vT_sb[:, jt, :, 0:64],
                        pv[:, 0:256].rearrange("p (h d) -> p h d", h=4))
                else:
                    nc.scalar.activation(
                        vT_sb[:, jt, :, 0:64],
                        pv[:, 0:256].rearrange("p (h d) -> p h d", h=4),
                        ActCopy)

            # ---- attention chunk: sims in groups of 2 j-tiles ----
            def attention_chunk(hp, off, cw, filler=None):
                po = PSO.tile([128, 1024], F32, tag="po", name="po")

                def sim_pair(jt, ps):
                    js = slice(jt * 128, (jt + 1) * 128)
                    nc.tensor.matmul(
                        ps[:, 0:cw],
                        mc(khat(hp)[0:64, js]),
                        mc(qhat(hp)[0:64, off:off + cw]),
                        start=True, stop=True)
                    nc.tensor.matmul(
                        ps[:, 512:512 + cw],
                        mc(khat(hp)[64:128, js]),
                        mc(qhat(hp)[64:128, off:off + cw]),
                        start=True, stop=True)

                def exp_jt(jt, ps, e):
                    ps3 = ps.rearrange("p (b c) -> p b c", b=2)
                    e3b = e.rearrange("p (b c) -> p b c", b=2)
                    if jt in DVE_JTS[hp]:
                        nc.vector._custom_dve(
                            expops["b16"],
                            out=e3b[:, :, 0:cw].bitcast(I16),
                            in0=ps3[:, :, 0:cw],
                            s0=A_E16, s1=B_E16)
                    else:
                        nc.scalar.activation(e3b[:, :, 0:cw],
                                             ps3[:, :, 0:cw], Exp)

                def ev_group(jt, e):
                    # 65-wide stationary: rows 0:64 = attn @ v, row 64 =
                    # softmax denominator (ones column in vT)
                    st, sp = (jt == 0), (jt == NJ - 1)
                    nc.tensor.matmul(
                        po[0:65, 0:cw],
                        mc(vT_sb[:, jt, 2 * hp, :]),
                        mc(e[:, 0:cw]),
                        start=st, stop=sp, skip_group_check=True)
                    nc.tensor.matmul(
                        po[0:65, 512:512 + cw],
                        mc(vT_sb[:, jt, 2 * hp + 1, :]),
                        mc(e[:, 512:512 + cw]),
                        start=st, stop=sp, skip_group_check=True)

                # per group of 2 j-tiles: 4 sims (alternating moving
                # partition halves -> PE overlaps them), the 2 exps, then
                # the trailing group's 4 E@v matmuls.
                pend = []
                for g in range((NJ + 3) // 4):
                    jts = [jt for jt in range(4 * g, 4 * g + 4) if jt < NJ]
                    tiles = []
                    for jt in jts:
                        ps = PSSIM.tile([128, 1024], F32, tag="ps",
                                        name="ps")
                        sim_pair(jt, ps)
                        tiles.append((jt, ps))
                    for jt, ps in tiles:
                        e = ESB.tile([128, 1024], WD, tag="e", name="e")
                        exp_jt(jt, ps, e)
                        pend.append((jt, e))
                    if filler is not None:
                        filler(g)
                    while len(pend) > 5:
                        j0, ee = pend.pop(0)
                        ev_group(j0, ee)
                for (j0, ee) in pend:
                    ev_group(j0, ee)
                # drain numerators + denominator row
                nc.vector.tensor_copy(numer[hp][0:64, off:off + cw],
                                      po[0:64, 0:cw])
                nc.vector.tensor_copy(numer[hp][64:128, off:off + cw],
                                      po[0:64, 512:512 + cw])
                dstt = s8 if hp == 0 else s8b
                nc.vector.tensor_copy(dstt[0:1, off:off + cw],
                                      po[64:65, 0:cw])
                nc.vector.tensor_copy(dstt[32:33, off:off + cw],
                                      po[64:65, 512:512 + cw])

            # ---- 1/s scaling ----
            def scale_chunk(hp, off, cw, src, rsbd, via_pe=False,
                            mul_eng=None):
                nc.vector.reciprocal_approx_fast(
                    out=rsden8[0:34, off:off + cw],
                    in_=src[0:34, off:off + cw])
                if via_pe:
                    # broadcast the two recip rows across partitions with
                    # K=1 bf16 matmuls (ones-row stationary) -- no DRAM
                    # round trip, so the tail chain after the last E@v
                    # stays short
                    nc.vector.tensor_copy(rsdenb[0:34, off:off + cw],
                                          rsden8[0:34, off:off + cw])
                    pbc = PSF.tile([128, 512], F32, tag="pf", name="pbc")
                    for t in range(2):
                        nc.tensor.matmul(
                            pbc[64 * t:64 * t + 64, 0:cw],
                            mc(ones8_sb[32 * t:32 * t + 1, 8:72]),
                            mc(rsdenb[32 * t:32 * t + 1, off:off + cw]),
                            start=True, stop=True, skip_group_check=True)
                    nc.scalar.activation(rsbd[:, off:off + cw],
                                         pbc[:, 0:cw], ActCopy)
                else:
                    t0 = 2 * hp
                    for t in range(2):
                        nc.sync.dma_start(
                            rsdd[t0 + t:t0 + t + 1, off:off + cw],
                            rsden8[32 * t:32 * t + 1, off:off + cw])
                    bcast_row(rsdd[t0][off:off + cw],
                              rsbd[0:64, off:off + cw], 64)
                    bcast_row(rsdd[t0 + 1][off:off + cw],
                              rsbd[64:128, off:off + cw], 64)
                (mul_eng or nc.vector).tensor_mul(
                    nsc[hp][:, off:off + cw],
                    numer[hp][:, off:off + cw],
                    rsbd[:, off:off + cw])

            # ---- merged output projection ----
            def outproj_chunk(off, cw):
                for m2 in range(2):
                    pf = PSF.tile([128, 512], F32, tag="pf", name="pf")
                    nc.tensor.matmul(
                        pf[:, 0:cw],
                        mc(woT_sb[0][:, m2 * 128:(m2 + 1) * 128]),
                        mc(nsc[0][:, off:off + cw]),
                        start=True, stop=False, skip_group_check=True)
                    nc.tensor.matmul(
                        pf[:, 0:cw],
                        mc(woT_sb[1][:, m2 * 128:(m2 + 1) * 128]),
                        mc(nsc[1][:, off:off + cw]),
                        start=False, stop=True, skip_group_check=True)
                    yt = YST.tile([128, 512], WD, tag="yt", name="yt")
                    nc.vector.tensor_scalar_add(
                        yt[:, 0:cw], pf[:, 0:cw], bias_sb[m2][:, :])
                    nc.sync.dma_start(y[m2][:, off:off + cw], yt[:, 0:cw])

            # ---- schedule ----
            rsb0p = RSB.tile([128, 2, N], WD, tag="rsb", name="rsb0p")
            rsb1p = RSB.tile([128, 2, N], WD, tag="rsb", name="rsb1p")

            # phase 1 in three stall-free PE passes: all pair-0 projection
            # matmuls (each waits only on its x-chunk DMA), all v^T tiles,
            # then the norm-sum matmuls (their squares computed long since)
            # with the rs/norm chains chasing chunk by chunk.
            for ci, (off, cw) in enumerate(CHUNKS):
                qkv_proj(0, off, cw, "act" if ci % 2 == 0 else "dve",
                         q2p0[ci], PSF, "pf")
            for jt in range(NJ):
                vproj(jt)
            for ci, (off, cw) in enumerate(CHUNKS):
                ss_sum(0, off, cw, "act" if ci % 2 == 0 else "dve",
                       q2p0[ci], PSF, "pf")
                rs_chunk(0, off, cw)
                norm_chunk(0, off, cw, rsb0p, nc.vector)

            attention_chunk(0, *CHUNKS[0])

            # scale broadcast tiles reuse the "rsb" slots: rsbd0 takes
            # rsb0p's buffer (its norm reads are all in phase 1), rsbd1
            # takes rsb1p's (reads end with hp0).
            rsbd0 = RSB.tile([128, N], F32, tag="rsb", name="rsbd0")
            rsbd1 = RSB.tile([128, N], F32, tag="rsb", name="rsbd1")

            # hp0 chunks 1-4 with pair-1 QKV+norms spread between them
            for ci, (off, cw) in enumerate(CHUNKS[1:], start=1):
                qkv_pair(1, *CHUNKS[ci - 1], "dve")
                rs_chunk(1, *CHUNKS[ci - 1])
                norm_chunk(1, *CHUNKS[ci - 1], rsb1p, nc.gpsimd)
                attention_chunk(0, off, cw)
                scale_chunk(0, *CHUNKS[ci - 1], s8, rsbd0,
                            mul_eng=nc.gpsimd)
            qkv_pair(1, *CHUNKS[-1], "dve")
            rs_chunk(1, *CHUNKS[-1])
            norm_chunk(1, *CHUNKS[-1], rsb1p, nc.gpsimd)
            scale_chunk(0, *CHUNKS[-1], s8, rsbd0, mul_eng=nc.gpsimd)

            # hp1 attention; pair-1 scaling + merged outproj pipelined one
            # chunk behind inside its window.
            for ci, (off, cw) in enumerate(CHUNKS):
                attention_chunk(1, off, cw)
                scale_chunk(1, off, cw, s8b, rsbd1, via_pe=True)
                if ci >= 1:
                    outproj_chunk(*CHUNKS[ci - 1])
            outproj_chunk(*CHUNKS[-1])

    nc.compile()
    return nc


def _get_program(wd_name=WD_NAME):
    if wd_name not in _CACHE:
        _CACHE[wd_name] = _build(wd_name)
    return _CACHE[wd_name]


def _np_wd(wd_name):
    if wd_name == "bf16":
        import ml_dtypes
        return np.dtype(ml_dtypes.bfloat16)
    return np.dtype(np.float32)


def make_in_maps(x, w_qkv, w_out, b_out, wd_name=WD_NAME):
    x = np.asarray(x, np.float32)
    w_qkv = np.asarray(w_qkv, np.float32)
    w_out = np.asarray(w_out, np.float32)
    b_out = np.asarray(b_out, np.float32)
    wd = _np_wd(wd_name)

    ones8 = np.zeros((128, 72), np.float32)
    for cc in range(8):
        lo = 64 * (cc % 2)
        ones8[lo:lo + 64, cc] = 1.0
    ones8[0, 8:72] = 1.0
    ones8[32, 8:72] = 1.0

    in_maps = []
    for core in range(8):
        b, half = core // 2, core % 2
        hsel = slice(256 * half, 256 * (half + 1))
        q_rows = np.arange(0, 512)[hsel]
        k_rows = 512 + q_rows
        v_rows = 1024 + q_rows
        wqk_h = np.ascontiguousarray(
            w_qkv[np.r_[q_rows, k_rows], :].T).reshape(2, 128, 512)
        wvT_h = np.ascontiguousarray(w_qkv[v_rows, :].T).reshape(2, 128, 256)
        woT_h = np.ascontiguousarray(w_out[:, hsel].T).reshape(2, 128, 256)
        bias_h = (b_out if half == 0 else np.zeros_like(b_out))
        in_maps.append({
            "x2": x[b].reshape(C, N).reshape(2, 128, N).astype(wd),
            "wqk": wqk_h.astype(wd),
            "wvT": wvT_h.astype(wd),
            "woT": woT_h.astype(wd),
            "bias": bias_h.reshape(2, 128, 1).astype(np.float32),
            "ones8": ones8.astype(wd),
        })
    return in_maps


def gather_output(results):
    outs = [np.asarray(r["y"], dtype=np.float32).reshape(C, N)
            for r in results]
    return np.stack([
        (outs[2 * b] + outs[2 * b + 1]).reshape(C, H, W) for b in range(B)
    ]).astype(np.float32)


def run(in_maps, wd_name=WD_NAME, **kwargs):
    from concourse import bass_utils
    nc = _get_program(wd_name)
    return bass_utils.run_bass_kernel_spmd(nc, in_maps,
                                           core_ids=list(range(8)), **kwargs)


def kernel(x, w_qkv, w_out, b_out):
    in_maps = make_in_maps(x, w_qkv, w_out, b_out)
    res = run(in_maps)
    return gather_output(res.results)


# revision 51
# speedup vs baseline: 1.0224x; 1.0224x over previous
"""Trainium2 Bass kernel for nn_Attention_21895743275585.

Reference computation (per batch b of 4):
  qkv = w_qkv @ x_flat            # 1x1 conv, x_flat [C=256, N=2304]
  q,k l2-normalized per (head, n) along dim_head=64; SCALE=10
  sim = 10 * qhat^T khat per head; attn = softmax(sim, axis=-1)
  out = attn @ v; final = w_out @ out_inner + b_out

Sharding: 8 cores = (batch b, head-half). Each core handles 4 of the 8 heads
of one batch; on-core the two head-pair output projections accumulate in
PSUM, so each core emits one [256, N] partial and the host sums the two
half-cores per batch (bias fed only to half 0).

On-core layout ([partition, free]):
  q,k "channels-major" [d, n]: qk4 [128, 4(t), N] with t in {q01,k01,q23,k23}
  v transposed [n, d] per j-tile with a ones column appended (65-wide
  stationary), so each E@v matmul also emits that head's softmax
  denominator row into PSUM partition 64 -- no separate ones matmuls.
  sim^T chunk [j, i] = k^T q; sims are emitted in groups of two j-tiles:
  consecutive sim matmuls stream moving data from alternating SBUF
  partition halves (q rows 0:64 / 64:128), which the PE overlaps; the
  four E@v matmuls of the group follow (full-partition moving, serial).
  softmax exp PSUM->SBUF splits between ACT (table Exp) and DVE (one-op
  Schraudolph: int16(x*184.665+16251) bitcast to bf16; numerator and
  denominator share the approximation so its ~3% jitter cancels in the
  softmax). 1/sqrt for the l2 norms runs as exp(-0.5 ln x) on ACT (one
  pinned table set); norm rows broadcast across partitions on the GPSIMD
  engine (no DRAM bounce), then one 2x-mode bf16 DVE multiply per (pair,
  chunk) forms qhat,khat packed [128, 4, N]. GPSIMD also takes the big
  memsets and the pair-1 squares/multiplies that sit inside the
  head-pair-0 attention window.
  Phase 1 (projection chunks 1-4 + v^T tiles) is interleaved INTO the
  first attention chunk's j-loop: the PE executes its queue in order, so
  emitting sims early keeps every engine fed during the latency-bound
  projection->norm chains. All projection work shares one PSUM pool
  (PSSIM 4 banks + PSO 2 + PSF 2 = 8).
"""

import math

import numpy as np

B, C, H, W = 4, 256, 48, 48
HEADS, DIM_HEAD, SCALE = 8, 64, 10.0
INNER = HEADS * DIM_HEAD
N = H * W                      # 2304
NJ = N // 128                  # 18 j-tiles
CHUNKS = [(0, 512), (512, 512), (1024, 512), (1536, 512), (2048, 256)]
EPS = 1e-12

WD_NAME = "bf16"               # working dtype: "bf16" | "f32r" | "f32"

# softmax-exp engine split: these j-tiles run on DVE (one-op Schraudolph),
# the rest on ACT (table exp). Tuned so ACT ~ DVE ~ just under PE per chunk.
DVE_JTS = {0: (2, 5, 8, 10, 12, 14, 16), 1: (2, 5, 8, 10, 12, 14, 16)}
LOG2E = 1.4426950408889634
# fp32-bits Schraudolph constants (legacy two-op path, kept registered)
A_EXP = float(2 ** 23) * LOG2E
B_EXP = float(127 * 2 ** 23) + 0.5
MASK_C = float(np.int32(0x007FFFFF).view(np.float32))
GAMMA = 0.235
# bf16-bits one-op Schraudolph: int16(x * 128*log2e + (127*128 + 0.5 - 5.51))
# the -5.51 centers the (1+f)/2^f interpolation error to +-3.0%.
A_E16 = 128.0 * LOG2E
B_E16 = 127.0 * 128.0 + 0.5 - 5.5085

_CACHE = {}


def _register_exp_ops():
    """Register the Schraudolph exp ops into concourse's custom-DVE tables
    (runtime registration; shas computed on the fly)."""
    import concourse.dve_ops as dops
    if "EXP_B16_ANT" in dops.CUSTOM_DVE_SPECS:
        return {"bits": dops._EXP_BITS_ANT, "fix": dops._EXP_FIX_ANT,
                "b16": dops._EXP_B16_ANT}
    from concourse.dve_spec import Spec, Src0, C0, C1, C2, AluOp, Bin, lower
    from concourse.dve_uop import DveOpSpec
    from concourse.dve_ops import DveOp

    def mk(name, spec):
        shas = {}
        for ver in ("v3", "v4"):
            try:
                sp = DveOpSpec(name=name, opcode=1,
                               uops=lower(spec, ver=ver), rd1_en=False)
                shas[ver] = sp.sha(ver)
            except Exception:
                pass
        op = DveOp(name, spec, subdim=False, uops_sha=shas)
        row = max(dops._SUB_OPCODE_FOR_NAME.values()) + 1
        assert row < 0x20
        dops.OPS.append(op)
        dops._SUB_OPCODE_FOR_NAME[op.name] = row
        dops.CUSTOM_DVE_SPECS[op.name] = op.spec
        return op

    def ref_bits(in0, in1, c0, c1, c2):
        t = in0.astype(np.float32) * np.float32(c0) + np.float32(c1)
        return t.astype(np.int32)

    spec_bits = Spec(body=Src0 * C0 + C1, reference=ref_bits)

    _and = Bin(AluOp.BITWISE_AND, Src0, C0)
    _u = Bin(AluOp.BITWISE_OR, _and, C1)
    _f = _u - C1
    _c = _f * (C1 - _f) * C2 + C1

    def ref_fix(in0, in1, c0, c1, c2):
        bits = np.asarray(in0, np.float32).view(np.int32)
        m = bits & 0x007FFFFF
        u = (m | 0x3F800000).astype(np.int32).view(np.float32)
        f = u - np.float32(c1)
        c = f * (np.float32(c1) - f) * np.float32(c2) + np.float32(c1)
        return np.asarray(in0, np.float32).view(np.float32) * c

    spec_fix = Spec(body=Src0 * _c, reference=ref_fix)

    def ref_b16(in0, in1, c0, c1, c2):
        # hardware: fp32 ALU result, output-stage convert to int16 (trunc);
        # CoreSim casts the returned float to the out AP dtype itself
        t = in0.astype(np.float32) * np.float32(c0) + np.float32(c1)
        return t

    spec_b16 = Spec(body=Src0 * C0 + C1, reference=ref_b16)

    dops._EXP_BITS_ANT = mk("EXP_BITS_ANT", spec_bits)
    dops._EXP_FIX_ANT = mk("EXP_FIX_ANT", spec_fix)
    dops._EXP_B16_ANT = mk("EXP_B16_ANT", spec_b16)
    return {"bits": dops._EXP_BITS_ANT, "fix": dops._EXP_FIX_ANT,
            "b16": dops._EXP_B16_ANT}


def _pin_act_tables():
    """Force every activation onto the natural_log_exp_and_others set so the
    whole kernel needs exactly one ACT table load (Ln+Exp share that set)."""
    import concourse.bacc as bacc_mod
    if getattr(bacc_mod, "_act_tables_pinned", False):
        return
    orig = bacc_mod.get_activation_tables

    def patched(arch):
        t = orig(arch)
        keep = "natural_log_exp_and_others"
        if keep in t:
            return {name: (funcs if name == keep else set())
                    for name, funcs in t.items()}
        return t

    bacc_mod.get_activation_tables = patched
    bacc_mod._act_tables_pinned = True


def _build(wd_name):
    import concourse.bass as bass
    import concourse.tile as tile
    from concourse import bacc, mybir

    _pin_act_tables()
    expops = _register_exp_ops()

    F32 = mybir.dt.float32
    I16 = mybir.dt.int16
    F32R = mybir.dt.float32r
    WD = mybir.dt.bfloat16 if wd_name == "bf16" else F32

    def mc(ap):
        # matmul operand cast for the fast-fp32 PE path
        return ap.bitcast(F32R) if wd_name == "f32r" else ap

    Ln = mybir.ActivationFunctionType.Ln
    Exp = mybir.ActivationFunctionType.Exp
    ActCopy = mybir.ActivationFunctionType.Copy

    nc = bacc.Bacc("TRN2", target_bir_lowering=False, debug=False,
                   enable_asserts=False, num_devices=8)
    x2 = nc.dram_tensor("x2", [2, 128, N], WD, kind="ExternalInput").ap()
    wqk = nc.dram_tensor("wqk", [2, 128, 512], WD, kind="ExternalInput").ap()
    wvT = nc.dram_tensor("wvT", [2, 128, 256], WD, kind="ExternalInput").ap()
    woT = nc.dram_tensor("woT", [2, 128, 256], WD, kind="ExternalInput").ap()
    bias = nc.dram_tensor("bias", [2, 128, 1], F32, kind="ExternalInput").ap()
    ones8 = nc.dram_tensor("ones8", [128, 72], WD, kind="ExternalInput").ap()
    # output: both head-pair projections pre-summed in PSUM; host adds the
    # two half-cores per batch. [m2, 128, N] row-blocks of out channels.
    y = nc.dram_tensor("y", [2, 128, N], WD, kind="ExternalOutput").ap()
    # internal DRAM bounce rows for the partition broadcasts (GPSIMD
    # partition_broadcast miscomputes on hardware here, so both the norm
    # and scale paths bounce through DRAM)
    rsd = nc.dram_tensor("rsd", [8, N], WD, kind="Internal").ap()
    rsdd = nc.dram_tensor("rsdd", [4, N], F32, kind="Internal").ap()

    def bcast_row(dram_row_ap, dst_ap, parts):
        src = bass.AP(tensor=dram_row_ap.tensor, offset=dram_row_ap.offset,
                      ap=[[0, parts]] + list(dram_row_ap.ap))
        nc.sync.dma_start(dst_ap, src)

    # pair-major tile order t: 0=q01, 1=k01, 2=q23, 3=k23.
    # wqk stationary column block for t:  m = [0, 2, 1, 3][t]
    T2M = [0, 2, 1, 3]

    with tile.TileContext(nc) as tc:
        with tc.tile_pool(name="persist", bufs=1) as P, \
             tc.tile_pool(name="bcast", bufs=2) as RSB, \
             tc.tile_pool(name="sq", bufs=3) as SQ, \
             tc.tile_pool(name="esb", bufs=12) as ESB, \
             tc.tile_pool(name="yst", bufs=3) as YST, \
             tc.tile_pool(name="pssim", bufs=2, space="PSUM") as PSSIM, \
             tc.tile_pool(name="pso", bufs=1, space="PSUM") as PSO, \
             tc.tile_pool(name="psf", bufs=2, space="PSUM") as PSF:

            # ---- persistent tiles ----
            x_sb = [P.tile([128, N], WD, tag=f"x{c}", name=f"x{c}")
                    for c in range(2)]
            wqk_sb = [P.tile([128, 512], WD, tag=f"wqk{c}", name=f"wqk{c}")
                      for c in range(2)]
            wvT_sb = [P.tile([128, 256], WD, tag=f"wvT{c}", name=f"wvT{c}")
                      for c in range(2)]
            woT_sb = [P.tile([128, 256], WD, tag=f"woT{c}", name=f"woT{c}")
                      for c in range(2)]
            bias_sb = [P.tile([128, 1], F32, tag=f"bias{c}", name=f"bias{c}")
                       for c in range(2)]
            ones8_sb = P.tile([128, 72], WD, tag="ones8", name="ones8")

            # PE warmup: two matmuls on a memset tile ramp the PE clock out
            # of its low-power state while the input DMAs are in flight.
            wu = P.tile([128, 64], WD, tag="wu", name="wu")
            nc.vector.memset(wu[:, :], 0.25)
            for _ in range(2):
                pwu = PSF.tile([128, 512], F32, tag="pf", name="pwu")
                nc.tensor.matmul(pwu[0:64, 0:64], mc(wu[:, :]),
                                 mc(wu[:, 0:64]), start=True, stop=True)

            # chunked input DMA: qk weights + chunk 0 of x first so the
            # first projection matmul can start early.
            nc.sync.dma_start(wqk_sb[0][:, :], wqk[0])
            for c in range(2):
                nc.sync.dma_start(x_sb[c][:, 0:256], x2[c][:, 0:256])
            for c in range(2):
                nc.sync.dma_start(x_sb[c][:, 256:512], x2[c][:, 256:512])
            nc.sync.dma_start(wqk_sb[1][:, :], wqk[1])
            for c in range(2):
                nc.sync.dma_start(wvT_sb[c][:, :], wvT[c])
            for (off, cw) in CHUNKS[1:]:
                for c in range(2):
                    nc.sync.dma_start(x_sb[c][:, off:off + cw],
                                      x2[c][:, off:off + cw])
            for c in range(2):
                nc.sync.dma_start(woT_sb[c][:, :], woT[c])
                nc.sync.dma_start(bias_sb[c][:, :], bias[c])
            nc.sync.dma_start(ones8_sb[:, :], ones8)

            # per-partition Exp bias: ln(SCALE) on q rows (bases 0, 64),
            # 0 on k rows (bases 32, 96)
            biasln = P.tile([128, 1], F32, tag="biasln", name="biasln")
            nc.vector.memset(biasln[0:32, :], math.log(SCALE))
            nc.vector.memset(biasln[32:64, :], 0.0)
            nc.vector.memset(biasln[64:96, :], math.log(SCALE))
            nc.vector.memset(biasln[96:128, :], 0.0)

            qk4 = P.tile([128, 4, N], WD, tag="qk4", name="qk4")
            ss8 = P.tile([128, N], F32, tag="ss8", name="ss8")
            rs8 = P.tile([128, N], WD, tag="rs8", name="rs8")
            nc.gpsimd.memset(ss8[:, :], 1.0)

            qh4 = P.tile([128, 4, N], WD, tag="qh4", name="qh4")
            vT_sb = P.tile([128, NJ, 4, 65], WD, tag="vT", name="vT")
            nc.gpsimd.memset(vT_sb[:, :, :, 64:65], 1.0)

            numer = [P.tile([128, N], WD, tag=f"nu{p}", name=f"nu{p}")
                     for p in range(2)]
            nsc = [P.tile([128, N], WD, tag=f"nsc{p}", name=f"nsc{p}")
                   for p in range(2)]
            # softmax denominators: head rows at partitions 0 and 32
            s8 = P.tile([64, N], F32, tag="s8", name="s8")
            s8b = P.tile([64, N], F32, tag="s8b", name="s8b")
            rsden8 = P.tile([64, N], F32, tag="rsden", name="rsden")
            rsdenb = P.tile([64, N], WD, tag="rsdenb", name="rsdenb")

            def qhat(p):
                return qh4[:, 2 * p, :]

            def khat(p):
                return qh4[:, 2 * p + 1, :]

            # ---- projection / norm / v^T building blocks (PSF pool) ----
            # q2 squares live per chunk until the deferred norm-sum pass;
            # they borrow the e-tile slots (same 2KB size, disjoint lifetime)
            q2p0 = [ESB.tile([128, 2, 512], WD, tag="e",
                             name=f"q2p{ci}") for ci in range(len(CHUNKS))]

            def qkv_proj(p, off, cw, copy_eng, q2, PQ, ptag):
                """project q and k tiles of pair p for one chunk + square.
                The norm-sum matmuls are deferred (ss_sum) so the PE queue
                never stalls on the drain->square chain."""
                for ti in range(2):
                    t = 2 * p + ti
                    m = T2M[t]
                    pq = PQ.tile([128, 512], F32, tag=ptag, name="pq")
                    for c in range(2):
                        nc.tensor.matmul(
                            pq[:, 0:cw],
                            mc(wqk_sb[c][:, m * 128:(m + 1) * 128]),
                            mc(x_sb[c][:, off:off + cw]),
                            start=(c == 0), stop=(c == 1))
                    if copy_eng == "act":
                        nc.scalar.activation(qk4[:, t, off:off + cw],
                                             pq[:, 0:cw], ActCopy)
                    else:
                        nc.vector.tensor_copy(qk4[:, t, off:off + cw],
                                              pq[:, 0:cw])
                sq_eng = nc.gpsimd
                sq_eng.tensor_mul(q2[:, :, 0:cw],
                                  qk4[:, 2 * p:2 * p + 2, off:off + cw],
                                  qk4[:, 2 * p:2 * p + 2, off:off + cw])

            def ss_sum(p, off, cw, copy_eng, q2, PQ, ptag):
                for ti in range(2):
                    base = 32 * (2 * p + ti)
                    pss = PQ.tile([128, 512], F32, tag=ptag, name="pss")
                    nc.tensor.matmul(pss[0:2, 0:cw],
                                     mc(ones8_sb[:, 0:2]),
                                     mc(q2[:, ti, 0:cw]),
                                     start=True, stop=True)
                    if copy_eng == "act":
                        nc.scalar.activation(
                            ss8[base:base + 2, off:off + cw],
                            pss[0:2, 0:cw], ActCopy)
                    else:
                        nc.vector.tensor_copy(
                            ss8[base:base + 2, off:off + cw],
                            pss[0:2, 0:cw])

            def qkv_pair(p, off, cw, copy_eng):
                q2 = SQ.tile([128, 2, 512], WD, tag="q2", name="q2")
                qkv_proj(p, off, cw, copy_eng, q2, PSF, "pf")
                ss_sum(p, off, cw, copy_eng, q2, PSF, "pf")

            def rs_chunk(p, off, cw):
                # rs = exp(-0.5 ln(ss) + biasln) on the packed norm rows
                b0 = 64 * p
                sl = slice(b0, b0 + 34)
                lnq = SQ.tile([64, 512], F32, tag="lnq", name="lnq")
                nc.scalar.activation(lnq[0:34, 0:cw], ss8[sl, off:off + cw],
                                     Ln)
                nc.scalar.activation(rs8[sl, off:off + cw],
                                     lnq[0:34, 0:cw], Exp,
                                     scale=-0.5, bias=biasln[sl, :])
                for a in (2 * p, 2 * p + 1):
                    nc.sync.dma_start(rsd[2 * a:2 * a + 2, off:off + cw],
                                      rs8[32 * a:32 * a + 2, off:off + cw])

            def norm_chunk(p, off, cw, rsbp, mul_eng):
                # rsbp [128, 2, N] bf16: [:,0,:] q-norm rows, [:,1,:] k
                for ti in range(2):
                    a = 2 * p + ti
                    bcast_row(rsd[2 * a][off:off + cw],
                              rsbp[0:64, ti, off:off + cw], 64)
                    bcast_row(rsd[2 * a + 1][off:off + cw],
                              rsbp[64:128, ti, off:off + cw], 64)
                mul_eng.tensor_mul(qh4[:, 2 * p:2 * p + 2, off:off + cw],
                                   qk4[:, 2 * p:2 * p + 2, off:off + cw],
                                   rsbp[:, :, off:off + cw])

            def vproj(jt):
                pv = PSF.tile([128, 512], F32, tag="pf", name="pv")
                for c in range(2):
                    nc.tensor.matmul(
                        pv[:, 0:256],
                        mc(x_sb[c][:, jt * 128:(jt + 1) * 128]),
                        mc(wvT_sb[c][:, :]),
                        start=(c == 0), stop=(c == 1))
                if jt % 3 != 1:
                    nc.vector.tensor_copy(
                        vT_sb[:, jt, :, 0:64],
                        pv[:, 0:256].rearrange("p (h d) -> p h d", h=4))
                else:
                    nc.scalar.activation(
                        vT_sb[:, jt, :, 0:64],
                        pv[:, 0:256].rearrange("p (h d) -> p h d", h=4),
                        ActCopy)

            # ---- attention chunk: sims in groups of 2 j-tiles ----
            def attention_chunk(hp, off, cw, filler=None):
                po = PSO.tile([128, 1024], F32, tag="po", name="po")

                def sim_pair(jt, ps):
                    js = slice(jt * 128, (jt + 1) * 128)
                    nc.tensor.matmul(
                        ps[:, 0:cw],
                        mc(khat(hp)[0:64, js]),
                        mc(qhat(hp)[0:64, off:off + cw]),
                        start=True, stop=True)
                    nc.tensor.matmul(
                        ps[:, 512:512 + cw],
                        mc(khat(hp)[64:128, js]),
                        mc(qhat(hp)[64:128, off:off + cw]),
                        start=True, stop=True)

                def exp_jt(jt, ps, e):
                    ps3 = ps.rearrange("p (b c) -> p b c", b=2)
                    e3b = e.rearrange("p (b c) -> p b c", b=2)
                    if jt in DVE_JTS[hp]:
                        nc.vector._custom_dve(
                            expops["b16"],
                            out=e3b[:, :, 0:cw].bitcast(I16),
                            in0=ps3[:, :, 0:cw],
                            s0=A_E16, s1=B_E16)
                    else:
                        nc.scalar.activation(e3b[:, :, 0:cw],
                                             ps3[:, :, 0:cw], Exp)

                def ev_group(jt, e):
                    # 65-wide stationary: rows 0:64 = attn @ v, row 64 =
                    # softmax denominator (ones column in vT)
                    st, sp = (jt == 0), (jt == NJ - 1)
                    nc.tensor.matmul(
                        po[0:65, 0:cw],
                        mc(vT_sb[:, jt, 2 * hp, :]),
                        mc(e[:, 0:cw]),
                        start=st, stop=sp, skip_group_check=True)
                    nc.tensor.matmul(
                        po[0:65, 512:512 + cw],
                        mc(vT_sb[:, jt, 2 * hp + 1, :]),
                        mc(e[:, 512:512 + cw]),
                        start=st, stop=sp, skip_group_check=True)

                # per group of 2 j-tiles: 4 sims (alternating moving
                # partition halves -> PE overlaps them), the 2 exps, then
                # the trailing group's 4 E@v matmuls.
                pend = []
                for g in range((NJ + 3) // 4):
                    jts = [jt for jt in range(4 * g, 4 * g + 4) if jt < NJ]
                    tiles = []
                    for jt in jts:
                        ps = PSSIM.tile([128, 1024], F32, tag="ps",
                                        name="ps")
                        sim_pair(jt, ps)
                        tiles.append((jt, ps))
                    for jt, ps in tiles:
                        e = ESB.tile([128, 1024], WD, tag="e", name="e")
                        exp_jt(jt, ps, e)
                        pend.append((jt, e))
                    if filler is not None:
                        filler(g)
                    while len(pend) > 5:
                        j0, ee = pend.pop(0)
                        ev_group(j0, ee)
                for (j0, ee) in pend:
                    ev_group(j0, ee)
                # drain numerators + denominator row
                nc.vector.tensor_copy(numer[hp][0:64, off:off + cw],
                                      po[0:64, 0:cw])
                nc.vector.tensor_copy(numer[hp][64:128, off:off + cw],
                                      po[0:64, 512:512 + cw])
                dstt = s8 if hp == 0 else s8b
                nc.vector.tensor_copy(dstt[0:1, off:off + cw],
                                      po[64:65, 0:cw])
                nc.vector.tensor_copy(dstt[32:33, off:off + cw],
                                      po[64:65, 512:512 + cw])

            # ---- 1/s scaling ----
            def scale_chunk(hp, off, cw, src, rsbd, via_pe=False,
                            mul_eng=None):
                nc.vector.reciprocal_approx_fast(
                    out=rsden8[0:34, off:off + cw],
                    in_=src[0:34, off:off + cw])
                if via_pe:
                    # broadcast the two recip rows across partitions with
                    # K=1 bf16 matmuls (ones-row stationary) -- no DRAM
                    # round trip, so the tail chain after the last E@v
                    # stays short
                    nc.vector.tensor_copy(rsdenb[0:34, off:off + cw],
                                          rsden8[0:34, off:off + cw])
                    pbc = PSF.tile([128, 512], F32, tag="pf", name="pbc")
                    for t in range(2):
                        nc.tensor.matmul(
                            pbc[64 * t:64 * t + 64, 0:cw],
                            mc(ones8_sb[32 * t:32 * t + 1, 8:72]),
                            mc(rsdenb[32 * t:32 * t + 1, off:off + cw]),
                            start=True, stop=True, skip_group_check=True)
                    nc.scalar.activation(rsbd[:, off:off + cw],
                                         pbc[:, 0:cw], ActCopy)
                else:
                    t0 = 2 * hp
                    for t in range(2):
                        nc.sync.dma_start(
                            rsdd[t0 + t:t0 + t + 1, off:off + cw],
                            rsden8[32 * t:32 * t + 1, off:off + cw])
                    bcast_row(rsdd[t0][off:off + cw],
                              rsbd[0:64, off:off + cw], 64)
                    bcast_row(rsdd[t0 + 1][off:off + cw],
                              rsbd[64:128, off:off + cw], 64)
                (mul_eng or nc.vector).tensor_mul(
                    nsc[hp][:, off:off + cw],
                    numer[hp][:, off:off + cw],
                    rsbd[:, off:off + cw])

            # ---- merged output projection ----
            def outproj_chunk(off, cw):
                for m2 in range(2):
                    pf = PSF.tile([128, 512], F32, tag="pf", name="pf")
                    nc.tensor.matmul(
                        pf[:, 0:cw],
                        mc(woT_sb[0][:, m2 * 128:(m2 + 1) * 128]),
                        mc(nsc[0][:, off:off + cw]),
                        start=True, stop=False, skip_group_check=True)
                    nc.tensor.matmul(
                        pf[:, 0:cw],
                        mc(woT_sb[1][:, m2 * 128:(m2 + 1) * 128]),
                        mc(nsc[1][:, off:off + cw]),
                        start=False, stop=True, skip_group_check=True)
                    yt = YST.tile([128, 512], WD, tag="yt", name="yt")
                    nc.vector.tensor_scalar_add(
                        yt[:, 0:cw], pf[:, 0:cw], bias_sb[m2][:, :])
                    nc.sync.dma_start(y[m2][:, off:off + cw], yt[:, 0:cw])

            # ---- schedule ----
            rsb0p = RSB.tile([128, 2, N], WD, tag="rsb", name="rsb0p")
            rsb1p = RSB.tile([128, 2, N], WD, tag="rsb", name="rsb1p")

            # phase 1 in three stall-free PE passes: all pair-0 projection
            # matmuls (each waits only on its x-chunk DMA), all v^T tiles,
            # then the norm-sum matmuls (their squares computed long since)
            # with the rs/norm chains chasing chunk by chunk.
            for ci, (off, cw) in enumerate(CHUNKS):
                qkv_proj(0, off, cw, "act" if ci % 2 == 0 else "dve",
                         q2p0[ci], PSF, "pf")
            for jt in range(NJ):
                vproj(jt)
            for ci, (off, cw) in enumerate(CHUNKS):
                ss_sum(0, off, cw, "act" if ci % 2 == 0 else "dve",
                       q2p0[ci], PSF, "pf")
                rs_chunk(0, off, cw)
                norm_chunk(0, off, cw, rsb0p, nc.vector)

            attention_chunk(0, *CHUNKS[0])

            # scale broadcast tiles reuse the "rsb" slots: rsbd0 takes
            # rsb0p's buffer (its norm reads are all in phase 1), rsbd1
            # takes rsb1p's (reads end with hp0).
            rsbd0 = RSB.tile([128, N], F32, tag="rsb", name="rsbd0")
            rsbd1 = RSB.tile([128, N], F32, tag="rsb", name="rsbd1")

            # hp0 chunks 1-4 with pair-1 QKV+norms spread between them
            for ci, (off, cw) in enumerate(CHUNKS[1:], start=1):
                qkv_pair(1, *CHUNKS[ci - 1], "dve")
                rs_chunk(1, *CHUNKS[ci - 1])
                norm_chunk(1, *CHUNKS[ci - 1], rsb1p, nc.gpsimd)
                attention_chunk(0, off, cw)
                scale_chunk(0, *CHUNKS[ci - 1], s8, rsbd0,
                            mul_eng=nc.gpsimd)
            qkv_pair(1, *CHUNKS[-1], "dve")
            rs_chunk(1, *CHUNKS[-1])
            norm_chunk(1, *CHUNKS[-1], rsb1p, nc.gpsimd)
            scale_chunk(0, *CHUNKS[-1], s8, rsbd0, mul_eng=nc.gpsimd)

            # hp1 attention; pair-1 scaling + merged outproj pipelined one
            # chunk behind inside its window.
            for ci, (off, cw) in enumerate(CHUNKS):
                attention_chunk(1, off, cw)
                scale_chunk(1, off, cw, s8b, rsbd1, via_pe=True)
                if ci >= 1:
                    outproj_chunk(*CHUNKS[ci - 1])
            outproj_chunk(*CHUNKS[-1])

    nc.compile()
    return nc


def _get_program(wd_name=WD_NAME):
    if wd_name not in _CACHE:
        _CACHE[wd_name] = _build(wd_name)
    return _CACHE[wd_name]


def _np_wd(wd_name):
    if wd_name == "bf16":
        import ml_dtypes
        return np.dtype(ml_dtypes.bfloat16)
    return np.dtype(np.float32)


def make_in_maps(x, w_qkv, w_out, b_out, wd_name=WD_NAME):
    x = np.asarray(x, np.float32)
    w_qkv = np.asarray(w_qkv, np.float32)
    w_out = np.asarray(w_out, np.float32)
    b_out = np.asarray(b_out, np.float32)
    wd = _np_wd(wd_name)

    ones8 = np.zeros((128, 72), np.float32)
    for cc in range(8):
        lo = 64 * (cc % 2)
        ones8[lo:lo + 64, cc] = 1.0
    ones8[0, 8:72] = 1.0
    ones8[32, 8:72] = 1.0

    in_maps = []
    for core in range(8):
        b, half = core // 2, core % 2
        hsel = slice(256 * half, 256 * (half + 1))
        q_rows = np.arange(0, 512)[hsel]
        k_rows = 512 + q_rows
        v_rows = 1024 + q_rows
        wqk_h = np.ascontiguousarray(
            w_qkv[np.r_[q_rows, k_rows], :].T).reshape(2, 128, 512)
        wvT_h = np.ascontiguousarray(w_qkv[v_rows, :].T).reshape(2, 128, 256)
        woT_h = np.ascontiguousarray(w_out[:, hsel].T).reshape(2, 128, 256)
        bias_h = (b_out if half == 0 else np.zeros_like(b_out))
        in_maps.append({
            "x2": x[b].reshape(C, N).reshape(2, 128, N).astype(wd),
            "wqk": wqk_h.astype(wd),
            "wvT": wvT_h.astype(wd),
            "woT": woT_h.astype(wd),
            "bias": bias_h.reshape(2, 128, 1).astype(np.float32),
            "ones8": ones8.astype(wd),
        })
    return in_maps


def gather_output(results):
    outs = [np.asarray(r["y"], dtype=np.float32).reshape(C, N)
            for r in results]
    return np.stack([
        (outs[2 * b] + outs[2 * b + 1]).reshape(C, H, W) for b in range(B)
    ]).astype(np.float32)


def run(in_maps, wd_name=WD_NAME, **kwargs):
    from concourse import bass_utils
    nc = _get_program(wd_name)
    return bass_utils.run_bass_kernel_spmd(nc, in_maps,
                                           core_ids=list(range(8)), **kwargs)


def kernel(x, w_qkv, w_out, b_out):
    in_maps = make_in_maps(x, w_qkv, w_out, b_out)
    res = run(in_maps)
    return gather_output(res.results)
